# revision 30
# baseline (speedup 1.0000x reference)
"""GraphSAGE fraud detector on 8 trn2 NeuronCores.

Strategy (dst-sharded graph parallel):
  - Nodes sharded across 8 cores (12500/core, padded to 12544 = 98*128).
  - Feature build: x/time on host-side layout, user/loc embedding gathers on
    device via dma_gather; time MLP via small matmuls.
  - Per layer: AllGather h (fp16) -> per-core edge gather (dma_gather from the
    gathered copy, 4 src windows of 25088 rows to fit int16 indices) ->
    segment-mean via one-hot matmuls accumulated in PSUM (mean folded into the
    one-hot as 1/deg) -> dense W_l/W_r matmuls + bias + relu.
  - Classifier: Wc matmul + sigmoid, output own shard, host concatenates.

All data-dependent static structure (per-(group,bucket) chunk counts) is made
uniform across the 8 cores by padding each segment to the max across cores, so
one SPMD program serves all cores.
"""

import sys

sys.path.insert(0, "/opt/trn_rl_repo")

import numpy as np

import concourse.bacc as bacc
import concourse.bass as bass
import concourse.mybir as mybir
import concourse.tile as tile
from concourse.bass_utils import run_bass_kernel_spmd
from concourse.library_config import mlp
from concourse.masks import make_identity

F16 = mybir.dt.float16
F32 = mybir.dt.float32
I16 = mybir.dt.int16

N = 100000
E = 1600000
C = 8
NV = 12500          # valid nodes per core
NPC = 12544         # padded nodes per core (98*128)
NG = 98             # dst groups of 128 per core
SG = 7              # groups per stage
NSTAGES = 14
NB = 4              # src buckets
W = 2 * NPC         # src window (25088 < 32768, int16-safe)
NPAD = C * NPC      # 100352 = 4*W
HID = 128


def _blocks():
    out = []
    w0 = 0
    while w0 < NPC:
        wd = min(512, NPC - w0)
        out.append((w0, wd))
        w0 += wd
    return out


def _time_chunks():
    # chunks of NPC//k, each a multiple of 128
    for tch in (1792, 512, 256, 128):
        if NPC % tch == 0:
            return tch, NPC // tch
    raise ValueError


def _build_structure(edge_index):
    """Host-side edge preprocessing. Returns per-core arrays + uniform layout."""
    src = np.asarray(edge_index[0], dtype=np.int64)
    dst = np.asarray(edge_index[1], dtype=np.int64)
    deg = np.bincount(dst, minlength=N).astype(np.float32)
    invdeg = 1.0 / np.maximum(deg, 1.0)

    owner = dst // NV
    l = (dst - owner * NV).astype(np.int64)          # local dst id
    g = l // 128                                      # dst group
    dcol = (l - g * 128).astype(np.float32)           # col within group
    sown = src // NV
    sp = sown * NPC + (src - sown * NV)               # padded global src id
    bkt = sp // W
    scol = (sp - bkt * W).astype(np.int64)            # index within window

    key = g * NB + bkt                                # (g, p) flat key
    counts = np.zeros((C, NG * NB), dtype=np.int64)
    per_core = []
    for c in range(C):
        m = owner == c
        kc = key[m]
        counts[c] = np.bincount(kc, minlength=NG * NB)
        per_core.append((kc, scol[m], dcol[m], invdeg[dst[m]]))

    Bseg = ((counts.max(axis=0) + 127) // 128) * 128  # uniform budgets [NG*NB]
    # equalize gather-call lengths across (stage, bucket) so the device
    # program needs a single num_idxs register value: pad the last group's
    # budget of each (s, p) so all stage-bucket sums match the global max.
    B2 = Bseg.reshape(NG, NB).copy()
    L_sp0 = np.array([[B2[s * SG:(s + 1) * SG, p].sum() for p in range(NB)]
                      for s in range(NSTAGES)])
    L_all = int(L_sp0.max())
    for s in range(NSTAGES):
        for p in range(NB):
            B2[s * SG + SG - 1, p] += L_all - L_sp0[s, p]
    Bseg = B2.reshape(NG * NB)
    # slot order: stage-major, then bucket, then group
    order_keys = []
    for s in range(NSTAGES):
        for p in range(NB):
            for gi in range(SG):
                order_keys.append((s * SG + gi) * NB + p)
    order_keys = np.array(order_keys)
    seg_len_ordered = Bseg[order_keys]
    seg_off_ordered = np.concatenate([[0], np.cumsum(seg_len_ordered)[:-1]])
    TOT = int(seg_len_ordered.sum())
    seg_off = np.zeros(NG * NB, dtype=np.int64)
    seg_off[order_keys] = seg_off_ordered

    core_arrays = []
    for c in range(C):
        kc, scol_c, dcol_c, inv_c = per_core[c]
        o = np.argsort(kc, kind="stable")
        kc_s = kc[o]
        # position within each key group
        cnt = counts[c]
        starts = np.concatenate([[0], np.cumsum(cnt)[:-1]])
        pos = np.arange(len(kc_s)) - starts[kc_s]
        slot = seg_off[kc_s] + pos
        idx16 = np.zeros(TOT, dtype=np.int16)
        dca = np.full(TOT, -1.0, dtype=np.float32)
        iva = np.zeros(TOT, dtype=np.float32)
        idx16[slot] = scol_c[o].astype(np.int16)
        dca[slot] = dcol_c[o]
        iva[slot] = inv_c[o]
        idx_arr = np.tile(idx16.reshape(TOT // 16, 16).T, (8, 1))  # [128, TOT/16]
        dc_arr = dca.reshape(TOT // 128, 128).T.astype(np.float32)  # [128, TOT/128]
        iv_arr = iva.reshape(TOT // 128, 128).T.astype(np.float32)
        core_arrays.append((np.ascontiguousarray(idx_arr),
                            np.ascontiguousarray(dc_arr),
                            np.ascontiguousarray(iv_arr)))
    return Bseg.reshape(NG, NB), seg_off.reshape(NG, NB), TOT, core_arrays


NWIN = 25           # dst windows per core: 24 x 512 cols + 1 x 256 cols
WIN_W = [512] * 24 + [256]
WIN_G0 = [w * 4 for w in range(25)]     # first 128-group of each window
NSTAGE2 = 5
STAGE_WINS = [list(range(s * 5, (s + 1) * 5)) for s in range(NSTAGE2)]


def _build_structure2(edge_index):
    """v2 edge prep: segments keyed by (dst 512-window, src bucket).

    Layout: stage-major -> bucket -> window; one uniform gather length L_all
    per (stage, bucket) call. Returns (B [NWIN,NB], seg_off [NWIN,NB], L_all,
    TOT, core_arrays).
    """
    src = np.asarray(edge_index[0], dtype=np.int64)
    dst = np.asarray(edge_index[1], dtype=np.int64)
    deg = np.bincount(dst, minlength=N).astype(np.float32)
    invdeg = 1.0 / np.maximum(deg, 1.0)

    owner = dst // NV
    l = (dst - owner * NV).astype(np.int64)
    w = np.minimum(l // 512, NWIN - 1)
    dcol = (l - w * 512).astype(np.float32)           # col within window
    sown = src // NV
    sp = sown * NPC + (src - sown * NV)
    bkt = sp // W
    scol = (sp - bkt * W).astype(np.int64)

    key = w * NB + bkt
    counts = np.zeros((C, NWIN * NB), dtype=np.int64)
    per_core = []
    for c in range(C):
        m = owner == c
        kc = key[m]
        counts[c] = np.bincount(kc, minlength=NWIN * NB)
        per_core.append((kc, scol[m], dcol[m], invdeg[dst[m]]))

    B2 = (((counts.max(axis=0) + 127) // 128) * 128).reshape(NWIN, NB)
    # equalize per-(stage,bucket) gather lengths to one global L_all by
    # padding the last window of each stage; the equalization pad is trailing
    # in its call, gets idx=-1, and the gather ucode (whose row count comes
    # from the per-call register = valid count) never generates descriptors
    # for it. Braw keeps the real (gathered) budgets for the matmul chunks.
    Braw = B2.copy()
    L_sp0 = np.array([[B2[s * 5:(s + 1) * 5, p].sum() for p in range(NB)]
                      for s in range(NSTAGE2)])
    L_all = int(L_sp0.max())
    for s in range(NSTAGE2):
        for p in range(NB):
            B2[s * 5 + 4, p] += L_all - L_sp0[s, p]
    # slot order: stage-major, then bucket, then window
    order_keys = []
    for s in range(NSTAGE2):
        for p in range(NB):
            for wi in STAGE_WINS[s]:
                order_keys.append(wi * NB + p)
    order_keys = np.array(order_keys)
    Bflat = B2.reshape(NWIN * NB)
    seg_len_ordered = Bflat[order_keys]
    seg_off_ordered = np.concatenate([[0], np.cumsum(seg_len_ordered)[:-1]])
    TOT = int(seg_len_ordered.sum())
    assert TOT == NSTAGE2 * NB * L_all
    seg_off = np.zeros(NWIN * NB, dtype=np.int64)
    seg_off[order_keys] = seg_off_ordered

    # equalization-pad slots (trailing in their call) are skipped by the
    # gather: idx=-1 and the per-call register count excludes them
    pad_mask = np.zeros(TOT, dtype=bool)
    for s in range(NSTAGE2):
        for p in range(NB):
            wl = s * 5 + 4
            o0 = seg_off[wl * NB + p] + Braw[wl, p]
            pad_mask[o0:o0 + (B2[wl, p] - Braw[wl, p])] = True

    core_arrays = []
    for c in range(C):
        kc, scol_c, dcol_c, inv_c = per_core[c]
        o = np.argsort(kc, kind="stable")
        kc_s = kc[o]
        cnt = counts[c]
        starts = np.concatenate([[0], np.cumsum(cnt)[:-1]])
        pos = np.arange(len(kc_s)) - starts[kc_s]
        slot = seg_off[kc_s] + pos
        idx16 = np.zeros(TOT, dtype=np.int16)
        idx16[pad_mask] = -1
        dca = np.full(TOT, -1.0, dtype=np.float32)
        iva = np.zeros(TOT, dtype=np.float32)
        idx16[slot] = scol_c[o].astype(np.int16)
        dca[slot] = dcol_c[o]
        iva[slot] = inv_c[o]
        idx_arr = np.tile(idx16.reshape(TOT // 16, 16).T, (8, 1))
        dc_arr = dca.reshape(TOT // 128, 128).T.astype(np.float32)
        iv_arr = iva.reshape(TOT // 128, 128).T.astype(np.float32)
        core_arrays.append((np.ascontiguousarray(idx_arr),
                            np.ascontiguousarray(dc_arr),
                            np.ascontiguousarray(iv_arr)))
    return B2, Braw, seg_off.reshape(NWIN, NB), L_all, TOT, core_arrays


def _build_nc2(B2, Braw, seg_off, L_all, TOT, use_cc=True, tr_gather=False):
    """v2 device program: 512-col dst windows, 5 stages x 4 bucket gather
    calls per layer, idx resident in SBUF across layers, bucket-major PSUM
    accumulation into one PSUM bank per window."""
    nc = bacc.Bacc("TRN2", num_devices=C)
    TOT16 = TOT // 16
    TOTC = TOT // 128

    p_idx = nc.declare_dram_parameter("idx_all", [128, TOT16], I16, isOutput=False)
    p_dc = nc.declare_dram_parameter("dc_all", [128, TOTC], F32, isOutput=False)
    p_iv = nc.declare_dram_parameter("iv_all", [128, TOTC], F32, isOutput=False)
    p_x = nc.declare_dram_parameter("x16", [NPC, 64], F16, isOutput=False)
    p_t5 = nc.declare_dram_parameter("timeT5", [5, NPC], F16, isOutput=False)
    p_uidx = nc.declare_dram_parameter("uidx", [128, NPC // 16], I16, isOutput=False)
    p_lidx = nc.declare_dram_parameter("lidx", [128, NPC // 16], I16, isOutput=False)
    p_ut = nc.declare_dram_parameter("utab", [10000, 128], F16, isOutput=False)
    p_lt = nc.declare_dram_parameter("ltab", [1000, 128], F16, isOutput=False)
    p_wt5 = nc.declare_dram_parameter("wt5", [5, 16], F16, isOutput=False)
    p_w = {}
    for nm in ("w1l", "w1r", "w2l", "w2r"):
        p_w[nm] = nc.declare_dram_parameter(nm, [128, 128], F16, isOutput=False)
    p_b1 = nc.declare_dram_parameter("b1", [128, 1], F32, isOutput=False)
    p_b2 = nc.declare_dram_parameter("b2", [128, 1], F32, isOutput=False)
    p_wc = nc.declare_dram_parameter("wc", [128, 1], F16, isOutput=False)
    p_bc = nc.declare_dram_parameter("bc", [1, 1], F32, isOutput=False)
    p_iota = nc.declare_dram_parameter("iota", [128, 512], F16, isOutput=False)
    p_out = nc.declare_dram_parameter("out", [1, NPC], F16, isOutput=True)

    cc_in = [nc.dram_tensor("cc_in0", [NPC, 128], F16),
             nc.dram_tensor("cc_in1", [NPC, 128], F16)]
    cc_out = [nc.dram_tensor("cc_out0", [NPAD, 128], F16, addr_space="Shared"),
              nc.dram_tensor("cc_out1", [NPAD, 128], F16, addr_space="Shared")]
    rg = [list(range(C))]

    O_sp = np.zeros((NSTAGE2, NB), dtype=np.int64)
    for s in range(NSTAGE2):
        for p in range(NB):
            O_sp[s, p] = seg_off[STAGE_WINS[s][0], p]

    from contextlib import ExitStack

    L_sp = np.array([[int(Braw[s * 5:(s + 1) * 5, p].sum()) for p in range(NB)]
                     for s in range(NSTAGE2)])

    with tile.TileContext(nc) as tc, ExitStack() as es:
        nc.gpsimd.load_library(mlp)
        _snap_cache = {}

        def snapv(v):
            if v not in _snap_cache:
                _snap_cache[v] = nc.gpsimd.snap(v)
            return _snap_cache[v]
        snapN = nc.gpsimd.snap(NPC)
        consts = es.enter_context(tc.tile_pool(name="consts", bufs=1))
        big = es.enter_context(tc.tile_pool(name="big", bufs=1))
        msgp = es.enter_context(tc.tile_pool(name="msgp", bufs=3))
        idxp = es.enter_context(tc.tile_pool(name="idxp", bufs=3))
        ohp = es.enter_context(tc.tile_pool(name="ohp", bufs=8))
        t5p = es.enter_context(tc.tile_pool(name="t5p", bufs=2))
        outp = es.enter_context(tc.tile_pool(name="outp", bufs=3))
        aggps = es.enter_context(tc.tile_pool(name="aggps", bufs=5, space="PSUM"))
        ps2p = es.enter_context(
            tc.tile_pool(name="ps2p", bufs=1 if tr_gather else 2, space="PSUM"))
        trp = es.enter_context(
            tc.tile_pool(name="trp", bufs=2 if tr_gather else 1, space="PSUM"))
        chkp = (es.enter_context(tc.tile_pool(name="chkp", bufs=4))
                if tr_gather else None)

        iota_s = consts.tile_from(p_iota[:, :])
        wts = {nm: consts.tile_from(p_w[nm][:, :], name=nm) for nm in p_w}
        wt5_s = consts.tile_from(p_wt5[:, :])
        b1_s = consts.tile_from(p_b1[:, :])
        b2_s = consts.tile_from(p_b2[:, :])
        wc_s = consts.tile_from(p_wc[:, :])
        bc_s = consts.tile_from(p_bc[:, :])
        uidx_s = consts.tile_from(p_uidx[:, :])
        lidx_s = consts.tile_from(p_lidx[:, :])
        dc_s = consts.tile_from(p_dc[:, :])
        iv_s = consts.tile_from(p_iv[:, :])
        ident = consts.tile([128, 128], F16)
        make_identity(nc, ident[:, :])

        hT_cur = big.tile([128, NPC], F16, tag="hT0", name="hT0")

        # ---- phase 0: build h0 (node-major) ----
        h_nm = big.tile([128, NG * 128], F16, tag="hnm")
        h3 = h_nm[:, :].rearrange("p (g d) -> p g d", d=128)
        nc.gpsimd.dma_gather(h3, p_ut[:, :], uidx_s[:, :], NPC, snapN, 128,
                             single_packet=False)
        lg = big.tile([128, NG * 128], F16, tag="aggT")
        lg3 = lg[:, :].rearrange("p (g d) -> p g d", d=128)
        nc.gpsimd.dma_gather(lg3, p_lt[:, :], lidx_s[:, :], NPC, snapN, 128,
                             single_packet=False)
        nc.vector.tensor_copy(h3[:, :, 96:112], lg3[:, :, 96:112])
        nc.sync.dma_start(
            out=h3[:, :, 0:64],
            in_=p_x[:, :].rearrange("(g p) d -> p g d", p=128),
        )
        TCH, TNCH = _time_chunks()
        for t in range(TNCH):
            t5 = t5p.tile([5, TCH], F16, tag="t5")
            nc.sync.dma_start(out=t5[:, :], in_=p_t5[:, t * TCH:(t + 1) * TCH])
            for gi in range(TCH // 128):
                gg = t * (TCH // 128) + gi
                ps_t = trp.tile([128, 16], F32, tag="tr")
                nc.tensor.matmul(ps_t[:, :], t5[:, gi * 128:(gi + 1) * 128],
                                 wt5_s[:, :], start=True, stop=True)
                nc.vector.tensor_copy(h3[:, gg, 112:128], ps_t[:, :])
        for gg in range(NG):
            ps_tr = trp.tile([128, 128], F16, tag="tr")
            nc.tensor.transpose(ps_tr[:, :], h3[:, gg, :], ident[:, :])
            nc.vector.tensor_copy(hT_cur[:, gg * 128:(gg + 1) * 128], ps_tr[:, :])
        nc.sync.dma_start(
            out=cc_in[0][:, :].rearrange("(g p) d -> p g d", p=128),
            in_=h3[:, :, :],
        )
        if use_cc:
            nc.gpsimd.collective_compute(
                "AllGather", mybir.AluOpType.bypass, replica_groups=rg,
                ins=[cc_in[0][:, :]], outs=[cc_out[0][:, :]],
            )
        else:
            nc.sync.dma_start(out=cc_out[0][0:NPC, :], in_=cc_in[0][:, :])

        # ---- conv layers ----
        for layer in range(2):
            wl = wts["w1l" if layer == 0 else "w2l"]
            wr = wts["w1r" if layer == 0 else "w2r"]
            bl = b1_s if layer == 0 else b2_s
            src_h = cc_out[layer]
            aggT = big.tile([128, NPC], F16, tag="aggT", name=f"aggT{layer}")
            for s in range(NSTAGE2):
                pss = {}
                for wi in STAGE_WINS[s]:
                    pss[wi] = aggps.tile([128, WIN_W[wi]], F32, tag="agg",
                                         name=f"agg{wi}")
                for p in range(NB):
                    O = int(O_sp[s, p])
                    it = idxp.tile([128, L_all // 16], I16, tag="idx")
                    nc.sync.dma_start(out=it[:, :],
                                      in_=p_idx[:, O // 16:(O + L_all) // 16])
                    mt = msgp.tile([128, L_all], F16, tag="msg")
                    if tr_gather:
                        nc.gpsimd.dma_gather(
                            mt[:, :].rearrange("p (b d) -> p b d", b=1),
                            src_h[p * W:(p + 1) * W, :],
                            it[:, :], L_all, snapv(int(L_sp[s, p])), 128,
                            transpose=True, single_packet=False,
                        )
                    else:
                        nc.gpsimd.dma_gather(
                            mt[:, :].rearrange("p (b d) -> p b d", d=128),
                            src_h[p * W:(p + 1) * W, :],
                            it[:, :], L_all, snapv(int(L_sp[s, p])), 128,
                            single_packet=False,
                        )
                    for wi in STAGE_WINS[s]:
                        wd = WIN_W[wi]
                        nb_wp = int(Braw[wi, p]) // 128
                        mcol0 = (int(seg_off[wi, p]) - O) // 128
                        ccol0 = int(seg_off[wi, p]) // 128
                        for cch in range(nb_wp):
                            oh = ohp.tile([128, wd], F16, tag="oh")
                            col = ccol0 + cch
                            nc.vector.tensor_scalar(
                                oh[:, :], iota_s[:, 0:wd],
                                dc_s[:, col:col + 1], iv_s[:, col:col + 1],
                                op0=mybir.AluOpType.is_equal,
                                op1=mybir.AluOpType.mult,
                            )
                            mc = mcol0 + cch
                            if tr_gather:
                                ps_tr = trp.tile([128, 128], F16, tag="tr",
                                                 name="trc")
                                nc.tensor.transpose(
                                    ps_tr[:, :],
                                    mt[:, mc * 128:(mc + 1) * 128],
                                    ident[:, :])
                                ck = chkp.tile([128, 128], F16, tag="ck")
                                nc.scalar.activation(
                                    ck[:, :], ps_tr[:, :],
                                    mybir.ActivationFunctionType.Copy,
                                    scale=1.0)
                                lhs = ck
                            else:
                                lhs = mt[:, mc * 128:(mc + 1) * 128]
                            nc.tensor.matmul(
                                pss[wi][:, :],
                                lhs[:, :] if tr_gather else lhs,
                                oh[:, :],
                                start=(p == 0 and cch == 0),
                                stop=(p == NB - 1 and cch == nb_wp - 1),
                            )
                for wi in STAGE_WINS[s]:
                    c0 = WIN_G0[wi] * 128
                    nc.vector.tensor_copy(aggT[:, c0:c0 + WIN_W[wi]],
                                          pss[wi][:, :])
            # dense: hT_next = relu(Wl^T aggT + Wr^T hT + b)
            hT_in = hT_cur
            hT_out = big.tile([128, NPC], F16,
                              tag="hT1" if layer == 0 else "hT0",
                              name=f"hTo{layer}")
            for (w0, wd) in _blocks():
                ps2 = ps2p.tile([128, wd], F32, tag="ps2", name="ps2")
                nc.tensor.matmul(ps2[:, :], wl[:, :], aggT[:, w0:w0 + wd],
                                 start=True, stop=False)
                nc.tensor.matmul(ps2[:, :], wr[:, :], hT_in[:, w0:w0 + wd],
                                 start=False, stop=True)
                nc.scalar.activation(hT_out[:, w0:w0 + wd], ps2[:, :],
                                     mybir.ActivationFunctionType.Relu,
                                     bias=bl[:, :], scale=1.0)
            if layer == 0:
                h_nm2 = big.tile([128, NG * 128], F16, tag="hnm")
                h23 = h_nm2[:, :].rearrange("p (g d) -> p g d", d=128)
                for gg in range(NG):
                    ps_tr = trp.tile([128, 128], F16, tag="tr")
                    nc.tensor.transpose(ps_tr[:, :],
                                        hT_out[:, gg * 128:(gg + 1) * 128],
                                        ident[:, :])
                    nc.vector.tensor_copy(h23[:, gg, :], ps_tr[:, :])
                nc.sync.dma_start(
                    out=cc_in[1][:, :].rearrange("(g p) d -> p g d", p=128),
                    in_=h23[:, :, :],
                )
                if use_cc:
                    nc.gpsimd.collective_compute(
                        "AllGather", mybir.AluOpType.bypass, replica_groups=rg,
                        ins=[cc_in[1][:, :]], outs=[cc_out[1][:, :]],
                    )
                else:
                    nc.sync.dma_start(out=cc_out[1][0:NPC, :], in_=cc_in[1][:, :])
            hT_cur = hT_out

        # ---- classifier ----
        h2T = hT_cur
        for (w0, wd) in _blocks():
            ps3 = ps2p.tile([1, wd], F32, tag="ps2", name="ps3")
            nc.tensor.matmul(ps3[:, :], wc_s[:, :], h2T[:, w0:w0 + wd],
                             start=True, stop=True)
            ot = outp.tile([1, wd], F16, tag="ot")
            nc.scalar.activation(ot[:, :], ps3[:, :],
                                 mybir.ActivationFunctionType.Sigmoid,
                                 bias=bc_s[0:1, 0:1], scale=1.0)
            nc.sync.dma_start(out=p_out[0:1, w0:w0 + wd], in_=ot[:, :])

    nc.compile()
    return nc


def _build_nc(Bseg, seg_off, TOT, use_cc=True, ablate=(), single_packet=False):
    ab = set(ablate)
    nc = bacc.Bacc("TRN2", num_devices=C)
    TOT16 = TOT // 16
    TOTC = TOT // 128

    # ---- parameters ----
    p_idx = nc.declare_dram_parameter("idx_all", [128, TOT16], I16, isOutput=False)
    p_dc = nc.declare_dram_parameter("dc_all", [128, TOTC], F32, isOutput=False)
    p_iv = nc.declare_dram_parameter("iv_all", [128, TOTC], F32, isOutput=False)
    p_x = nc.declare_dram_parameter("x16", [NPC, 64], F16, isOutput=False)
    p_t5 = nc.declare_dram_parameter("timeT5", [5, NPC], F16, isOutput=False)
    p_uidx = nc.declare_dram_parameter("uidx", [128, NPC // 16], I16, isOutput=False)
    p_lidx = nc.declare_dram_parameter("lidx", [128, NPC // 16], I16, isOutput=False)
    p_ut = nc.declare_dram_parameter("utab", [10000, 128], F16, isOutput=False)
    p_lt = nc.declare_dram_parameter("ltab", [1000, 128], F16, isOutput=False)
    p_wt5 = nc.declare_dram_parameter("wt5", [5, 16], F16, isOutput=False)
    p_w = {}
    for nm in ("w1l", "w1r", "w2l", "w2r"):
        p_w[nm] = nc.declare_dram_parameter(nm, [128, 128], F16, isOutput=False)
    p_b1 = nc.declare_dram_parameter("b1", [128, 1], F32, isOutput=False)
    p_b2 = nc.declare_dram_parameter("b2", [128, 1], F32, isOutput=False)
    p_wc = nc.declare_dram_parameter("wc", [128, 1], F16, isOutput=False)
    p_bc = nc.declare_dram_parameter("bc", [1, 1], F32, isOutput=False)
    p_iota = nc.declare_dram_parameter("iota", [128, 128], F16, isOutput=False)
    p_out = nc.declare_dram_parameter("out", [1, NPC], F16, isOutput=True)

    cc_in = [nc.dram_tensor("cc_in0", [NPC, 128], F16),
             nc.dram_tensor("cc_in1", [NPC, 128], F16)]
    cc_out = [nc.dram_tensor("cc_out0", [NPAD, 128], F16, addr_space="Shared"),
              nc.dram_tensor("cc_out1", [NPAD, 128], F16, addr_space="Shared")]

    rg = [list(range(C))]

    # stage gather call layout
    L_sp = np.zeros((NSTAGES, NB), dtype=np.int64)
    O_sp = np.zeros((NSTAGES, NB), dtype=np.int64)
    for s in range(NSTAGES):
        for p in range(NB):
            L_sp[s, p] = Bseg[s * SG:(s + 1) * SG, p].sum()
            O_sp[s, p] = seg_off[s * SG, p]

    from contextlib import ExitStack

    with tile.TileContext(nc) as tc, ExitStack() as es:
        nc.gpsimd.load_library(mlp)
        _snap_cache = {}

        def snapv(v):
            if v not in _snap_cache:
                _snap_cache[v] = nc.gpsimd.snap(v)
            return _snap_cache[v]
        consts = es.enter_context(tc.tile_pool(name="consts", bufs=1))
        big = es.enter_context(tc.tile_pool(name="big", bufs=1))
        idxp = es.enter_context(tc.tile_pool(name="idxp", bufs=5))
        msgp = es.enter_context(tc.tile_pool(name="msgp", bufs=5))
        ohp = es.enter_context(tc.tile_pool(name="ohp", bufs=6))
        t5p = es.enter_context(tc.tile_pool(name="t5p", bufs=2))
        outp = es.enter_context(tc.tile_pool(name="outp", bufs=3))
        aggps = es.enter_context(tc.tile_pool(name="aggps", bufs=4, space="PSUM"))
        ps2p = es.enter_context(tc.tile_pool(name="ps2p", bufs=2, space="PSUM"))
        trp = es.enter_context(tc.tile_pool(name="trp", bufs=2, space="PSUM"))

        # ---- constants to SBUF ----
        iota_s = consts.tile_from(p_iota[:, :])
        wts = {nm: consts.tile_from(p_w[nm][:, :], name=nm) for nm in p_w}
        wt5_s = consts.tile_from(p_wt5[:, :])
        b1_s = consts.tile_from(p_b1[:, :])
        b2_s = consts.tile_from(p_b2[:, :])
        wc_s = consts.tile_from(p_wc[:, :])
        bc_s = consts.tile_from(p_bc[:, :])
        uidx_s = consts.tile_from(p_uidx[:, :])
        lidx_s = consts.tile_from(p_lidx[:, :])
        dc_s = consts.tile_from(p_dc[:, :])
        iv_s = consts.tile_from(p_iv[:, :])
        ident = consts.tile([128, 128], F16)
        make_identity(nc, ident[:, :])

        hT_cur = big.tile([128, NPC], F16, tag="hT0", name="hT0")
        aggT = big.tile([128, NPC], F16, tag="aggT")

        # ---- phase 0: build h0 (node-major) ----
        h_nm = big.tile([128, NG * 128], F16, tag="hnm")
        h3 = h_nm[:, :].rearrange("p (g d) -> p g d", d=128)
        # user emb gather straight into h_nm (table cols 64:96 hold the emb)
        nc.gpsimd.dma_gather(h3, p_ut[:, :], uidx_s[:, :], NPC, snapv(NPC), 128, single_packet=False)
        # loc emb gather to scratch (aggT slot reused), copy cols 96:112
        lg = big.tile([128, NG * 128], F16, tag="aggT")
        lg3 = lg[:, :].rearrange("p (g d) -> p g d", d=128)
        nc.gpsimd.dma_gather(lg3, p_lt[:, :], lidx_s[:, :], NPC, snapv(NPC), 128, single_packet=False)
        nc.vector.tensor_copy(h3[:, :, 96:112], lg3[:, :, 96:112])
        # x -> cols 0:64
        nc.sync.dma_start(
            out=h3[:, :, 0:64],
            in_=p_x[:, :].rearrange("(g p) d -> p g d", p=128),
        )
        # time mlp -> cols 112:128
        TCH, TNCH = _time_chunks()
        for t in range(TNCH):
            t5 = t5p.tile([5, TCH], F16, tag="t5")
            nc.sync.dma_start(out=t5[:, :], in_=p_t5[:, t * TCH:(t + 1) * TCH])
            for gi in range(TCH // 128):
                gg = t * (TCH // 128) + gi
                ps_t = trp.tile([128, 16], F32, tag="tr")
                nc.tensor.matmul(ps_t[:, :], t5[:, gi * 128:(gi + 1) * 128],
                                 wt5_s[:, :], start=True, stop=True)
                nc.vector.tensor_copy(h3[:, gg, 112:128], ps_t[:, :])
        # hT0 via PE transpose
        for gg in range(NG):
            ps_tr = trp.tile([128, 128], F16, tag="tr")
            nc.tensor.transpose(ps_tr[:, :], h3[:, gg, :], ident[:, :])
            nc.vector.tensor_copy(hT_cur[:, gg * 128:(gg + 1) * 128], ps_tr[:, :])
        # ship h0 to collective input
        nc.sync.dma_start(
            out=cc_in[0][:, :].rearrange("(g p) d -> p g d", p=128),
            in_=h3[:, :, :],
        )
        if use_cc:
            nc.gpsimd.collective_compute(
                "AllGather", mybir.AluOpType.bypass, replica_groups=rg,
                ins=[cc_in[0][:, :]], outs=[cc_out[0][:, :]],
            )
        else:
            nc.sync.dma_start(out=cc_out[0][0:NPC, :], in_=cc_in[0][:, :])

        # ---- conv layers ----
        for layer in range(2):
            wl = wts["w1l" if layer == 0 else "w2l"]
            wr = wts["w1r" if layer == 0 else "w2r"]
            bl = b1_s if layer == 0 else b2_s
            src_h = cc_out[layer]
            for s in range(NSTAGES):
                msgs = {}
                for p in range(NB):
                    L = int(L_sp[s, p])
                    if L == 0:
                        continue
                    O = int(O_sp[s, p])
                    it = idxp.tile([128, L // 16], I16, tag="idx")
                    if "idxdma" not in ab:
                        nc.sync.dma_start(out=it[:, :],
                                          in_=p_idx[:, O // 16:(O + L) // 16])
                    mt = msgp.tile([128, (L // 128) * 128], F16, tag="msg")
                    if "gather" not in ab:
                        nc.gpsimd.dma_gather(
                            mt[:, :].rearrange("p (b d) -> p b d", d=128),
                            src_h[p * W:(p + 1) * W, :],
                            it[:, :], L, snapv(L), 128,
                            single_packet=single_packet,
                        )
                    else:
                        nc.vector.tensor_copy(mt[:, 0:1], ident[:, 0:1])
                    msgs[p] = mt
                for gi in range(SG):
                    gg = s * SG + gi
                    nchunks = int(Bseg[gg, :].sum()) // 128
                    ps = aggps.tile([128, 128], F32, tag="agg")
                    k = 0
                    for p in range(NB):
                        nb_gp = int(Bseg[gg, p]) // 128
                        if nb_gp == 0:
                            continue
                        mt = msgs[p]
                        mcol0 = (int(seg_off[gg, p]) - int(O_sp[s, p])) // 128
                        ccol0 = int(seg_off[gg, p]) // 128
                        for cch in range(nb_gp):
                            oh = ohp.tile([128, 128], F16, tag="oh")
                            col = ccol0 + cch
                            if "onehot" not in ab:
                                nc.vector.tensor_scalar(
                                    oh[:, :], iota_s[:, :],
                                    dc_s[:, col:col + 1], iv_s[:, col:col + 1],
                                    op0=mybir.AluOpType.is_equal,
                                    op1=mybir.AluOpType.mult,
                                )
                            else:
                                nc.vector.tensor_copy(oh[:, 0:1], ident[:, 0:1])
                            mc = mcol0 + cch
                            if "mm" not in ab:
                                nc.tensor.matmul(
                                    ps[:, :], mt[:, mc * 128:(mc + 1) * 128],
                                    oh[:, :], start=(k == 0), stop=(k == nchunks - 1),
                                )
                            elif k == 0:
                                nc.tensor.matmul(
                                    ps[:, :], ident[:, :], ident[:, :],
                                    start=True, stop=True,
                                )
                            k += 1
                    nc.vector.tensor_copy(aggT[:, gg * 128:(gg + 1) * 128], ps[:, :])
            # dense: hT_next = relu(Wl^T aggT + Wr^T hT + b)
            hT_in = hT_cur
            hT_out = big.tile([128, NPC], F16,
                              tag="hT1" if layer == 0 else "hT0",
                              name=f"hTo{layer}")
            for (w0, wd) in _blocks():
                ps2 = ps2p.tile([128, wd], F32, tag="ps2", name="ps2")
                nc.tensor.matmul(ps2[:, :], wl[:, :], aggT[:, w0:w0 + wd],
                                 start=True, stop=False)
                nc.tensor.matmul(ps2[:, :], wr[:, :], hT_in[:, w0:w0 + wd],
                                 start=False, stop=True)
                nc.scalar.activation(hT_out[:, w0:w0 + wd], ps2[:, :],
                                     mybir.ActivationFunctionType.Relu,
                                     bias=bl[:, :], scale=1.0)
            if layer == 0:
                h_nm2 = big.tile([128, NG * 128], F16, tag="hnm")
                h23 = h_nm2[:, :].rearrange("p (g d) -> p g d", d=128)
                for gg in range(NG):
                    ps_tr = trp.tile([128, 128], F16, tag="tr")
                    nc.tensor.transpose(ps_tr[:, :],
                                        hT_out[:, gg * 128:(gg + 1) * 128],
                                        ident[:, :])
                    nc.vector.tensor_copy(h23[:, gg, :], ps_tr[:, :])
                nc.sync.dma_start(
                    out=cc_in[1][:, :].rearrange("(g p) d -> p g d", p=128),
                    in_=h23[:, :, :],
                )
                if use_cc:
                    nc.gpsimd.collective_compute(
                        "AllGather", mybir.AluOpType.bypass, replica_groups=rg,
                        ins=[cc_in[1][:, :]], outs=[cc_out[1][:, :]],
                    )
                else:
                    nc.sync.dma_start(out=cc_out[1][0:NPC, :], in_=cc_in[1][:, :])
            hT_cur = hT_out

        # ---- classifier ----
        h2T = hT_cur
        for (w0, wd) in _blocks():
            ps3 = ps2p.tile([1, wd], F32, tag="ps2", name="ps3")
            nc.tensor.matmul(ps3[:, :], wc_s[:, :], h2T[:, w0:w0 + wd],
                             start=True, stop=True)
            ot = outp.tile([1, wd], F16, tag="ot")
            nc.scalar.activation(ot[:, :], ps3[:, :],
                                 mybir.ActivationFunctionType.Sigmoid,
                                 bias=bc_s[0:1, 0:1], scale=1.0)
            nc.sync.dma_start(out=p_out[0:1, w0:w0 + wd], in_=ot[:, :])

    nc.compile()
    return nc


_CACHE = {}


def _pjrt_timed_runner(nc, n_cores):
    """Build a jitted SPMD executor for `nc` (same lowering path as
    bass_utils.run_bass_kernel_spmd -> bass2jax.run_bass_via_pjrt under axon),
    but with the jit object cached so repeat calls skip trace/compile.

    Returns run(in_maps, timed_iters) -> (results, exec_ns):
      results: list (per core) of {out_name: np.ndarray}
      exec_ns: min wall-ns of a warmed steady-state SPMD dispatch+execute
               (block_until_ready on device outputs; excludes H2D of inputs
               and NEFF compile).
    """
    import time as _time

    import jax
    from jax.experimental.shard_map import shard_map
    from jax.sharding import Mesh, NamedSharding, PartitionSpec

    from concourse import bass2jax
    from concourse.bass2jax import _bass_exec_p, partition_id_tensor

    bass2jax.install_neuronx_cc_hook()

    partition_name = nc.partition_id_tensor.name if nc.partition_id_tensor else None

    in_names = []
    out_names = []
    out_avals = []
    zero_outs = []
    for alloc in nc.m.functions[0].allocations:
        if not isinstance(alloc, mybir.MemoryLocationSet):
            continue
        name = alloc.memorylocations[0].name
        if alloc.kind == "ExternalInput":
            if name != partition_name:
                in_names.append(name)
        elif alloc.kind == "ExternalOutput":
            shape = tuple(alloc.tensor_shape)
            dtype = mybir.dt.np(alloc.dtype)
            out_avals.append(jax.core.ShapedArray(shape, dtype))
            zero_outs.append(np.zeros((n_cores * shape[0],) + shape[1:], dtype))
            out_names.append(name)
    n_params = len(in_names)
    n_outs = len(out_names)
    all_in_names = list(in_names) + list(out_names)
    if partition_name is not None:
        all_in_names.append(partition_name)
    donate = tuple(range(n_params, n_params + n_outs))

    def _body(*args):
        operands = list(args)
        if partition_name is not None:
            operands.append(partition_id_tensor())
        outs = _bass_exec_p.bind(
            *operands,
            out_avals=tuple(out_avals),
            in_names=tuple(all_in_names),
            out_names=tuple(out_names),
            lowering_input_output_aliases=(),
            sim_require_finite=True,
            sim_require_nnan=True,
            nc=nc,
        )
        return tuple(outs)

    devices = jax.devices()[:n_cores]
    mesh = Mesh(np.asarray(devices), ("core",))
    shard = NamedSharding(mesh, PartitionSpec("core"))
    in_specs = (PartitionSpec("core"),) * (n_params + n_outs)
    out_specs = (PartitionSpec("core"),) * n_outs
    sharded = jax.jit(
        shard_map(_body, mesh=mesh, in_specs=in_specs, out_specs=out_specs,
                  check_rep=False),
        donate_argnums=donate,
        keep_unused=True,
    )

    def run(in_maps, timed_iters=4):
        if nc.dbg_addr is not None:
            in_maps = [
                {**m, nc.dbg_addr.name: np.zeros((1, 2), np.uint32)}
                for m in in_maps
            ]
        concat_in = [
            np.concatenate([np.asarray(in_maps[c][name]) for c in range(n_cores)],
                           axis=0)
            for name in in_names
        ]
        in_dev = [jax.device_put(a, shard) for a in concat_in]
        jax.block_until_ready(in_dev)

        # warmup: triggers trace + NEFF compile + load; result reused as output
        zeros_dev = [jax.device_put(z, shard) for z in zero_outs]
        jax.block_until_ready(zeros_dev)
        out_arrs = sharded(*in_dev, *zeros_dev)
        jax.block_until_ready(out_arrs)

        # Timing: executions dispatched async pipeline through the axon
        # tunnel and serialize on-device, so the marginal cost of one more
        # execution is the true per-execution device time. Measure wall for
        # K1 and K2 back-to-back dispatch batches; slope = HW exec time
        # (tunnel round-trip latency cancels).
        def _batch_wall(k):
            zs = [[jax.device_put(z, shard) for z in zero_outs]
                  for _ in range(k)]
            for zl in zs:
                jax.block_until_ready(zl)
            t0 = _time.perf_counter()
            outs = [sharded(*in_dev, *zl) for zl in zs]
            jax.block_until_ready(outs)
            t1 = _time.perf_counter()
            return t1 - t0, outs

        K1, K2 = 4, 36
        exec_ns = None
        for _ in range(timed_iters):
            w1, _o1 = _batch_wall(K1)
            w2, _o2 = _batch_wall(K2)
            ns = int((w2 - w1) / (K2 - K1) * 1e9)
            exec_ns = ns if exec_ns is None else min(exec_ns, ns)

        results = []
        host = [np.asarray(a) for a in out_arrs]
        for c in range(n_cores):
            results.append(
                {name: host[i].reshape((n_cores,) + tuple(out_avals[i].shape))[c]
                 for i, name in enumerate(out_names)}
            )
        return results, exec_ns

    return run


def kernel(**inputs):
    x = np.asarray(inputs["x"], dtype=np.float32)
    edge_index = np.asarray(inputs["edge_index"])
    user_ids = np.asarray(inputs["user_ids"], dtype=np.int64)
    locations = np.asarray(inputs["locations"], dtype=np.int64)
    tf = np.asarray(inputs["time_features"], dtype=np.float32)

    B2, Braw, seg_off, L_all, TOT, core_arrays = _build_structure2(edge_index)

    key = ("nc2", TOT, tuple(B2.flatten().tolist()))
    if key not in _CACHE:
        _CACHE.clear()
        import os
        nc = _build_nc2(B2, Braw, seg_off, L_all, TOT,
                        use_cc=os.environ.get('NO_CC', '0') != '1')
        _CACHE[key] = (nc, _pjrt_timed_runner(nc, C))
    nc, runner = _CACHE[key]

    # shared (replicated) arrays
    ut = np.zeros((10000, 128), dtype=np.float16)
    ut[:, 64:96] = np.asarray(inputs["user_emb_table"], dtype=np.float32)
    lt = np.zeros((1000, 128), dtype=np.float16)
    lt[:, 96:112] = np.asarray(inputs["loc_emb_table"], dtype=np.float32)
    wt5 = np.concatenate(
        [np.asarray(inputs["W_time"], dtype=np.float32),
         np.asarray(inputs["b_time"], dtype=np.float32)[None, :]], axis=0
    ).astype(np.float16)
    iota = np.tile(np.arange(512, dtype=np.float16)[None, :], (128, 1))
    shared = {
        "utab": ut, "ltab": lt, "wt5": wt5, "iota": iota,
        "w1l": np.asarray(inputs["W1_l"], dtype=np.float16),
        "w1r": np.asarray(inputs["W1_r"], dtype=np.float16),
        "w2l": np.asarray(inputs["W2_l"], dtype=np.float16),
        "w2r": np.asarray(inputs["W2_r"], dtype=np.float16),
        "b1": np.asarray(inputs["b1"], dtype=np.float32).reshape(128, 1),
        "b2": np.asarray(inputs["b2"], dtype=np.float32).reshape(128, 1),
        "wc": np.asarray(inputs["Wc"], dtype=np.float16).reshape(128, 1),
        "bc": np.asarray(inputs["bc"], dtype=np.float32).reshape(1, 1),
    }

    in_maps = []
    for c in range(C):
        idx_arr, dc_arr, iv_arr = core_arrays[c]
        x16 = np.zeros((NPC, 64), dtype=np.float16)
        x16[:NV] = x[c * NV:(c + 1) * NV]
        t5 = np.ones((5, NPC), dtype=np.float16)
        t5[:4, :NV] = tf[c * NV:(c + 1) * NV].T
        t5[:4, NV:] = 0.0
        uid = np.zeros(NPC, dtype=np.int16)
        uid[:NV] = user_ids[c * NV:(c + 1) * NV]
        lid = np.zeros(NPC, dtype=np.int16)
        lid[:NV] = locations[c * NV:(c + 1) * NV]
        uidx = np.tile(uid.reshape(NPC // 16, 16).T, (8, 1))
        lidx = np.tile(lid.reshape(NPC // 16, 16).T, (8, 1))
        m = {
            "idx_all": idx_arr, "dc_all": dc_arr, "iv_all": iv_arr,
            "x16": x16, "timeT5": t5,
            "uidx": np.ascontiguousarray(uidx),
            "lidx": np.ascontiguousarray(lidx),
        }
        m.update(shared)
        in_maps.append(m)

    try:
        results, exec_ns = runner(in_maps)
        print(f"HW exec time: {exec_ns} ns")
    except Exception:
        import time as _time
        _t0 = _time.perf_counter()
        res = run_bass_kernel_spmd(nc, in_maps, list(range(C)))
        _t1 = _time.perf_counter()
        results = res.results
        print(f"HW exec time: {int((_t1 - _t0) * 1e9)} ns (wall of spmd call, upper bound)")
    out = np.zeros((N, 1), dtype=np.float32)
    for c in range(C):
        o = np.asarray(results[c]["out"], dtype=np.float32).reshape(NPC)
        out[c * NV:(c + 1) * NV, 0] = o[:NV]
    return out



# revision 32
# speedup vs baseline: 1.0430x; 1.0430x over previous
"""GraphSAGE fraud detector on 8 trn2 NeuronCores.

Strategy (dst-sharded graph parallel):
  - Nodes sharded across 8 cores (12500/core, padded to 12544 = 98*128).
  - Feature build: x/time on host-side layout, user/loc embedding gathers on
    device via dma_gather; time MLP via small matmuls.
  - Per layer: AllGather h (fp16) -> per-core edge gather (dma_gather from the
    gathered copy, 4 src windows of 25088 rows to fit int16 indices) ->
    segment-mean via one-hot matmuls accumulated in PSUM (mean folded into the
    one-hot as 1/deg) -> dense W_l/W_r matmuls + bias + relu.
  - Classifier: Wc matmul + sigmoid, output own shard, host concatenates.

All data-dependent static structure (per-(group,bucket) chunk counts) is made
uniform across the 8 cores by padding each segment to the max across cores, so
one SPMD program serves all cores.
"""

import sys

sys.path.insert(0, "/opt/trn_rl_repo")

import numpy as np

import concourse.bacc as bacc
import concourse.bass as bass
import concourse.mybir as mybir
import concourse.tile as tile
from concourse.bass_utils import run_bass_kernel_spmd
from concourse.library_config import mlp
from concourse.masks import make_identity

F16 = mybir.dt.float16
F32 = mybir.dt.float32
I16 = mybir.dt.int16

N = 100000
E = 1600000
C = 8
NV = 12500          # valid nodes per core
NPC = 12544         # padded nodes per core (98*128)
NG = 98             # dst groups of 128 per core
SG = 7              # groups per stage
NSTAGES = 14
NB = 4              # src buckets
W = 2 * NPC         # src window (25088 < 32768, int16-safe)
NPAD = C * NPC      # 100352 = 4*W
HID = 128


def _blocks():
    out = []
    w0 = 0
    while w0 < NPC:
        wd = min(512, NPC - w0)
        out.append((w0, wd))
        w0 += wd
    return out


def _time_chunks():
    # chunks of NPC//k, each a multiple of 128
    for tch in (1792, 512, 256, 128):
        if NPC % tch == 0:
            return tch, NPC // tch
    raise ValueError


def _build_structure(edge_index):
    """Host-side edge preprocessing. Returns per-core arrays + uniform layout."""
    src = np.asarray(edge_index[0], dtype=np.int64)
    dst = np.asarray(edge_index[1], dtype=np.int64)
    deg = np.bincount(dst, minlength=N).astype(np.float32)
    invdeg = 1.0 / np.maximum(deg, 1.0)

    owner = dst // NV
    l = (dst - owner * NV).astype(np.int64)          # local dst id
    g = l // 128                                      # dst group
    dcol = (l - g * 128).astype(np.float32)           # col within group
    sown = src // NV
    sp = sown * NPC + (src - sown * NV)               # padded global src id
    bkt = sp // W
    scol = (sp - bkt * W).astype(np.int64)            # index within window

    key = g * NB + bkt                                # (g, p) flat key
    counts = np.zeros((C, NG * NB), dtype=np.int64)
    per_core = []
    for c in range(C):
        m = owner == c
        kc = key[m]
        counts[c] = np.bincount(kc, minlength=NG * NB)
        per_core.append((kc, scol[m], dcol[m], invdeg[dst[m]]))

    Bseg = ((counts.max(axis=0) + 127) // 128) * 128  # uniform budgets [NG*NB]
    # equalize gather-call lengths across (stage, bucket) so the device
    # program needs a single num_idxs register value: pad the last group's
    # budget of each (s, p) so all stage-bucket sums match the global max.
    B2 = Bseg.reshape(NG, NB).copy()
    L_sp0 = np.array([[B2[s * SG:(s + 1) * SG, p].sum() for p in range(NB)]
                      for s in range(NSTAGES)])
    L_all = int(L_sp0.max())
    for s in range(NSTAGES):
        for p in range(NB):
            B2[s * SG + SG - 1, p] += L_all - L_sp0[s, p]
    Bseg = B2.reshape(NG * NB)
    # slot order: stage-major, then bucket, then group
    order_keys = []
    for s in range(NSTAGES):
        for p in range(NB):
            for gi in range(SG):
                order_keys.append((s * SG + gi) * NB + p)
    order_keys = np.array(order_keys)
    seg_len_ordered = Bseg[order_keys]
    seg_off_ordered = np.concatenate([[0], np.cumsum(seg_len_ordered)[:-1]])
    TOT = int(seg_len_ordered.sum())
    seg_off = np.zeros(NG * NB, dtype=np.int64)
    seg_off[order_keys] = seg_off_ordered

    core_arrays = []
    for c in range(C):
        kc, scol_c, dcol_c, inv_c = per_core[c]
        o = np.argsort(kc, kind="stable")
        kc_s = kc[o]
        # position within each key group
        cnt = counts[c]
        starts = np.concatenate([[0], np.cumsum(cnt)[:-1]])
        pos = np.arange(len(kc_s)) - starts[kc_s]
        slot = seg_off[kc_s] + pos
        idx16 = np.zeros(TOT, dtype=np.int16)
        dca = np.full(TOT, -1.0, dtype=np.float32)
        iva = np.zeros(TOT, dtype=np.float32)
        idx16[slot] = scol_c[o].astype(np.int16)
        dca[slot] = dcol_c[o]
        iva[slot] = inv_c[o]
        idx_arr = np.tile(idx16.reshape(TOT // 16, 16).T, (8, 1))  # [128, TOT/16]
        dc_arr = dca.reshape(TOT // 128, 128).T.astype(np.float32)  # [128, TOT/128]
        iv_arr = iva.reshape(TOT // 128, 128).T.astype(np.float32)
        core_arrays.append((np.ascontiguousarray(idx_arr),
                            np.ascontiguousarray(dc_arr),
                            np.ascontiguousarray(iv_arr)))
    return Bseg.reshape(NG, NB), seg_off.reshape(NG, NB), TOT, core_arrays


NWIN = 25           # dst windows per core: 24 x 512 cols + 1 x 256 cols
WIN_W = [512] * 24 + [256]
WIN_G0 = [w * 4 for w in range(25)]     # first 128-group of each window
NSTAGE2 = 5
STAGE_WINS = [list(range(s * 5, (s + 1) * 5)) for s in range(NSTAGE2)]


def _build_structure2(edge_index):
    """v2 edge prep: segments keyed by (dst 512-window, src bucket).

    Layout: stage-major -> bucket -> window; one uniform gather length L_all
    per (stage, bucket) call. Returns (B [NWIN,NB], seg_off [NWIN,NB], L_all,
    TOT, core_arrays).
    """
    src = np.asarray(edge_index[0], dtype=np.int64)
    dst = np.asarray(edge_index[1], dtype=np.int64)
    deg = np.bincount(dst, minlength=N).astype(np.float32)
    invdeg = 1.0 / np.maximum(deg, 1.0)

    owner = dst // NV
    l = (dst - owner * NV).astype(np.int64)
    w = np.minimum(l // 512, NWIN - 1)
    dcol = (l - w * 512).astype(np.float32)           # col within window
    sown = src // NV
    sp = sown * NPC + (src - sown * NV)
    bkt = sp // W
    scol = (sp - bkt * W).astype(np.int64)

    key = w * NB + bkt
    counts = np.zeros((C, NWIN * NB), dtype=np.int64)
    per_core = []
    for c in range(C):
        m = owner == c
        kc = key[m]
        counts[c] = np.bincount(kc, minlength=NWIN * NB)
        per_core.append((kc, scol[m], dcol[m], invdeg[dst[m]]))

    B2 = (((counts.max(axis=0) + 127) // 128) * 128).reshape(NWIN, NB)
    # equalize per-(stage,bucket) gather lengths to one global L_all by
    # padding the last window of each stage; the equalization pad is trailing
    # in its call, gets idx=-1, and the gather ucode (whose row count comes
    # from the per-call register = valid count) never generates descriptors
    # for it. Braw keeps the real (gathered) budgets for the matmul chunks.
    Braw = B2.copy()
    L_sp0 = np.array([[B2[s * 5:(s + 1) * 5, p].sum() for p in range(NB)]
                      for s in range(NSTAGE2)])
    L_all = int(L_sp0.max())
    for s in range(NSTAGE2):
        for p in range(NB):
            B2[s * 5 + 4, p] += L_all - L_sp0[s, p]
    # slot order: stage-major, then bucket, then window
    order_keys = []
    for s in range(NSTAGE2):
        for p in range(NB):
            for wi in STAGE_WINS[s]:
                order_keys.append(wi * NB + p)
    order_keys = np.array(order_keys)
    Bflat = B2.reshape(NWIN * NB)
    seg_len_ordered = Bflat[order_keys]
    seg_off_ordered = np.concatenate([[0], np.cumsum(seg_len_ordered)[:-1]])
    TOT = int(seg_len_ordered.sum())
    assert TOT == NSTAGE2 * NB * L_all
    seg_off = np.zeros(NWIN * NB, dtype=np.int64)
    seg_off[order_keys] = seg_off_ordered

    # equalization-pad slots (trailing in their call) are skipped by the
    # gather: idx=-1 and the per-call register count excludes them
    pad_mask = np.zeros(TOT, dtype=bool)
    for s in range(NSTAGE2):
        for p in range(NB):
            wl = s * 5 + 4
            o0 = seg_off[wl * NB + p] + Braw[wl, p]
            pad_mask[o0:o0 + (B2[wl, p] - Braw[wl, p])] = True

    core_arrays = []
    for c in range(C):
        kc, scol_c, dcol_c, inv_c = per_core[c]
        o = np.argsort(kc, kind="stable")
        kc_s = kc[o]
        cnt = counts[c]
        starts = np.concatenate([[0], np.cumsum(cnt)[:-1]])
        pos = np.arange(len(kc_s)) - starts[kc_s]
        slot = seg_off[kc_s] + pos
        idx16 = np.zeros(TOT, dtype=np.int16)
        idx16[pad_mask] = -1
        dca = np.full(TOT, -1.0, dtype=np.float32)
        iva = np.zeros(TOT, dtype=np.float32)
        idx16[slot] = scol_c[o].astype(np.int16)
        dca[slot] = dcol_c[o]
        iva[slot] = inv_c[o]
        idx_arr = np.tile(idx16.reshape(TOT // 16, 16).T, (8, 1))
        dc_arr = dca.reshape(TOT // 128, 128).T.astype(np.float32)
        iv_arr = iva.reshape(TOT // 128, 128).T.astype(np.float32)
        core_arrays.append((np.ascontiguousarray(idx_arr),
                            np.ascontiguousarray(dc_arr),
                            np.ascontiguousarray(iv_arr)))
    return B2, Braw, seg_off.reshape(NWIN, NB), L_all, TOT, core_arrays


def _build_nc2(B2, Braw, seg_off, L_all, TOT, use_cc=True, tr_gather=False):
    """v2 device program: 512-col dst windows, 5 stages x 4 bucket gather
    calls per layer, idx resident in SBUF across layers, bucket-major PSUM
    accumulation into one PSUM bank per window."""
    nc = bacc.Bacc("TRN2", num_devices=C)
    TOT16 = TOT // 16
    TOTC = TOT // 128

    p_idx = nc.declare_dram_parameter("idx_all", [128, TOT16], I16, isOutput=False)
    p_dc = nc.declare_dram_parameter("dc_all", [128, TOTC], F32, isOutput=False)
    p_iv = nc.declare_dram_parameter("iv_all", [128, TOTC], F32, isOutput=False)
    p_x = nc.declare_dram_parameter("x16", [NPC, 64], F16, isOutput=False)
    p_t5 = nc.declare_dram_parameter("timeT5", [5, NPC], F16, isOutput=False)
    p_uidx = nc.declare_dram_parameter("uidx", [128, NPC // 16], I16, isOutput=False)
    p_lidx = nc.declare_dram_parameter("lidx", [128, NPC // 16], I16, isOutput=False)
    p_ut = nc.declare_dram_parameter("utab", [10000, 128], F16, isOutput=False)
    p_lt = nc.declare_dram_parameter("ltab", [1000, 128], F16, isOutput=False)
    p_wt5 = nc.declare_dram_parameter("wt5", [5, 16], F16, isOutput=False)
    p_w = {}
    for nm in ("w1l", "w1r", "w2l", "w2r"):
        p_w[nm] = nc.declare_dram_parameter(nm, [128, 128], F16, isOutput=False)
    p_b1 = nc.declare_dram_parameter("b1", [128, 1], F32, isOutput=False)
    p_b2 = nc.declare_dram_parameter("b2", [128, 1], F32, isOutput=False)
    p_wc = nc.declare_dram_parameter("wc", [128, 1], F16, isOutput=False)
    p_bc = nc.declare_dram_parameter("bc", [1, 1], F32, isOutput=False)
    p_iota = nc.declare_dram_parameter("iota", [128, 512], F16, isOutput=False)
    p_out = nc.declare_dram_parameter("out", [1, NPC], F16, isOutput=True)

    cc_in = [nc.dram_tensor("cc_in0", [NPC, 128], F16),
             nc.dram_tensor("cc_in1", [NPC, 128], F16)]
    cc_out = [nc.dram_tensor("cc_out0", [NPAD, 128], F16, addr_space="Shared"),
              nc.dram_tensor("cc_out1", [NPAD, 128], F16, addr_space="Shared")]
    rg = [list(range(C))]

    O_sp = np.zeros((NSTAGE2, NB), dtype=np.int64)
    for s in range(NSTAGE2):
        for p in range(NB):
            O_sp[s, p] = seg_off[STAGE_WINS[s][0], p]

    from contextlib import ExitStack

    L_sp = np.array([[int(Braw[s * 5:(s + 1) * 5, p].sum()) for p in range(NB)]
                     for s in range(NSTAGE2)])

    with tile.TileContext(nc) as tc, ExitStack() as es:
        nc.gpsimd.load_library(mlp)
        _snap_cache = {}

        def snapv(v):
            if v not in _snap_cache:
                _snap_cache[v] = nc.gpsimd.snap(v)
            return _snap_cache[v]
        snapN = nc.gpsimd.snap(NPC)
        consts = es.enter_context(tc.tile_pool(name="consts", bufs=1))
        big = es.enter_context(tc.tile_pool(name="big", bufs=1))
        msgp = es.enter_context(tc.tile_pool(name="msgp", bufs=3))
        idxp = es.enter_context(tc.tile_pool(name="idxp", bufs=3))
        ohp = es.enter_context(tc.tile_pool(name="ohp", bufs=8))
        t5p = es.enter_context(tc.tile_pool(name="t5p", bufs=2))
        outp = es.enter_context(tc.tile_pool(name="outp", bufs=3))
        aggps = es.enter_context(tc.tile_pool(name="aggps", bufs=5, space="PSUM"))
        ps2p = es.enter_context(
            tc.tile_pool(name="ps2p", bufs=1 if tr_gather else 2, space="PSUM"))
        trp = es.enter_context(
            tc.tile_pool(name="trp", bufs=2 if tr_gather else 1, space="PSUM"))
        chkp = (es.enter_context(tc.tile_pool(name="chkp", bufs=4))
                if tr_gather else None)

        iota_s = consts.tile_from(p_iota[:, :])
        wts = {nm: consts.tile_from(p_w[nm][:, :], name=nm) for nm in p_w}
        wt5_s = consts.tile_from(p_wt5[:, :])
        b1_s = consts.tile_from(p_b1[:, :])
        b2_s = consts.tile_from(p_b2[:, :])
        wc_s = consts.tile_from(p_wc[:, :])
        bc_s = consts.tile_from(p_bc[:, :])
        uidx_s = consts.tile_from(p_uidx[:, :])
        lidx_s = consts.tile_from(p_lidx[:, :])
        dc_s = consts.tile_from(p_dc[:, :])
        iv_s = consts.tile_from(p_iv[:, :])
        ident = consts.tile([128, 128], F16)
        make_identity(nc, ident[:, :])

        hT_cur = big.tile([128, NPC], F16, tag="hT0", name="hT0")

        # ---- phase 0: build h0 (node-major) ----
        h_nm = big.tile([128, NG * 128], F16, tag="hnm")
        h3 = h_nm[:, :].rearrange("p (g d) -> p g d", d=128)
        nc.gpsimd.dma_gather(h3, p_ut[:, :], uidx_s[:, :], NPC, snapN, 128,
                             single_packet=False)
        lg = big.tile([128, NG * 128], F16, tag="aggT")
        lg3 = lg[:, :].rearrange("p (g d) -> p g d", d=128)
        nc.gpsimd.dma_gather(lg3, p_lt[:, :], lidx_s[:, :], NPC, snapN, 128,
                             single_packet=False)
        nc.vector.tensor_copy(h3[:, :, 96:112], lg3[:, :, 96:112])
        nc.sync.dma_start(
            out=h3[:, :, 0:64],
            in_=p_x[:, :].rearrange("(g p) d -> p g d", p=128),
        )
        TCH, TNCH = _time_chunks()
        for t in range(TNCH):
            t5 = t5p.tile([5, TCH], F16, tag="t5")
            nc.sync.dma_start(out=t5[:, :], in_=p_t5[:, t * TCH:(t + 1) * TCH])
            for gi in range(TCH // 128):
                gg = t * (TCH // 128) + gi
                ps_t = trp.tile([128, 16], F32, tag="tr")
                nc.tensor.matmul(ps_t[:, :], t5[:, gi * 128:(gi + 1) * 128],
                                 wt5_s[:, :], start=True, stop=True)
                nc.vector.tensor_copy(h3[:, gg, 112:128], ps_t[:, :])
        for gg in range(NG):
            ps_tr = trp.tile([128, 128], F16, tag="tr")
            nc.tensor.transpose(ps_tr[:, :], h3[:, gg, :], ident[:, :])
            nc.vector.tensor_copy(hT_cur[:, gg * 128:(gg + 1) * 128], ps_tr[:, :])
        nc.sync.dma_start(
            out=cc_in[0][:, :].rearrange("(g p) d -> p g d", p=128),
            in_=h3[:, :, :],
        )
        if use_cc:
            nc.gpsimd.collective_compute(
                "AllGather", mybir.AluOpType.bypass, replica_groups=rg,
                ins=[cc_in[0][:, :]], outs=[cc_out[0][:, :]],
            )
        else:
            nc.sync.dma_start(out=cc_out[0][0:NPC, :], in_=cc_in[0][:, :])

        # ---- conv layers ----
        for layer in range(2):
            wl = wts["w1l" if layer == 0 else "w2l"]
            wr = wts["w1r" if layer == 0 else "w2r"]
            bl = b1_s if layer == 0 else b2_s
            src_h = cc_out[layer]
            aggT = big.tile([128, NPC], F16, tag="aggT", name=f"aggT{layer}")
            for s in range(NSTAGE2):
                pss = {}
                for wi in STAGE_WINS[s]:
                    pss[wi] = aggps.tile([128, WIN_W[wi]], F32, tag="agg",
                                         name=f"agg{wi}")
                for p in range(NB):
                    O = int(O_sp[s, p])
                    it = idxp.tile([128, L_all // 16], I16, tag="idx")
                    nc.sync.dma_start(out=it[:, :],
                                      in_=p_idx[:, O // 16:(O + L_all) // 16])
                    mt = msgp.tile([128, L_all], F16, tag="msg")
                    if tr_gather:
                        nc.gpsimd.dma_gather(
                            mt[:, :].rearrange("p (b d) -> p b d", b=1),
                            src_h[p * W:(p + 1) * W, :],
                            it[:, :], L_all, snapv(int(L_sp[s, p])), 128,
                            transpose=True, single_packet=False,
                        )
                    else:
                        nc.gpsimd.dma_gather(
                            mt[:, :].rearrange("p (b d) -> p b d", d=128),
                            src_h[p * W:(p + 1) * W, :],
                            it[:, :], L_all, snapv(int(L_sp[s, p])), 128,
                            single_packet=False,
                        )
                    for wi in STAGE_WINS[s]:
                        wd = WIN_W[wi]
                        nb_wp = int(Braw[wi, p]) // 128
                        mcol0 = (int(seg_off[wi, p]) - O) // 128
                        ccol0 = int(seg_off[wi, p]) // 128
                        for cch in range(nb_wp):
                            oh = ohp.tile([128, wd], F16, tag="oh")
                            col = ccol0 + cch
                            nc.vector.tensor_scalar(
                                oh[:, :], iota_s[:, 0:wd],
                                dc_s[:, col:col + 1], iv_s[:, col:col + 1],
                                op0=mybir.AluOpType.is_equal,
                                op1=mybir.AluOpType.mult,
                            )
                            mc = mcol0 + cch
                            if tr_gather:
                                ps_tr = trp.tile([128, 128], F16, tag="tr",
                                                 name="trc")
                                nc.tensor.transpose(
                                    ps_tr[:, :],
                                    mt[:, mc * 128:(mc + 1) * 128],
                                    ident[:, :])
                                ck = chkp.tile([128, 128], F16, tag="ck")
                                nc.scalar.activation(
                                    ck[:, :], ps_tr[:, :],
                                    mybir.ActivationFunctionType.Copy,
                                    scale=1.0)
                                lhs = ck
                            else:
                                lhs = mt[:, mc * 128:(mc + 1) * 128]
                            nc.tensor.matmul(
                                pss[wi][:, :],
                                lhs[:, :] if tr_gather else lhs,
                                oh[:, :],
                                start=(p == 0 and cch == 0),
                                stop=(p == NB - 1 and cch == nb_wp - 1),
                            )
                for wi in STAGE_WINS[s]:
                    c0 = WIN_G0[wi] * 128
                    nc.vector.tensor_copy(aggT[:, c0:c0 + WIN_W[wi]],
                                          pss[wi][:, :])
            # dense: hT_next = relu(Wl^T aggT + Wr^T hT + b)
            hT_in = hT_cur
            hT_out = big.tile([128, NPC], F16,
                              tag="hT1" if layer == 0 else "hT0",
                              name=f"hTo{layer}")
            for (w0, wd) in _blocks():
                ps2 = ps2p.tile([128, wd], F32, tag="ps2", name="ps2")
                nc.tensor.matmul(ps2[:, :], wl[:, :], aggT[:, w0:w0 + wd],
                                 start=True, stop=False)
                nc.tensor.matmul(ps2[:, :], wr[:, :], hT_in[:, w0:w0 + wd],
                                 start=False, stop=True)
                nc.scalar.activation(hT_out[:, w0:w0 + wd], ps2[:, :],
                                     mybir.ActivationFunctionType.Relu,
                                     bias=bl[:, :], scale=1.0)
            if layer == 0:
                h_nm2 = big.tile([128, NG * 128], F16, tag="hnm")
                h23 = h_nm2[:, :].rearrange("p (g d) -> p g d", d=128)
                for gg in range(NG):
                    ps_tr = trp.tile([128, 128], F16, tag="tr")
                    nc.tensor.transpose(ps_tr[:, :],
                                        hT_out[:, gg * 128:(gg + 1) * 128],
                                        ident[:, :])
                    nc.vector.tensor_copy(h23[:, gg, :], ps_tr[:, :])
                nc.sync.dma_start(
                    out=cc_in[1][:, :].rearrange("(g p) d -> p g d", p=128),
                    in_=h23[:, :, :],
                )
                if use_cc:
                    nc.gpsimd.collective_compute(
                        "AllGather", mybir.AluOpType.bypass, replica_groups=rg,
                        ins=[cc_in[1][:, :]], outs=[cc_out[1][:, :]],
                    )
                else:
                    nc.sync.dma_start(out=cc_out[1][0:NPC, :], in_=cc_in[1][:, :])
            hT_cur = hT_out

        # ---- classifier ----
        h2T = hT_cur
        for (w0, wd) in _blocks():
            ps3 = ps2p.tile([1, wd], F32, tag="ps2", name="ps3")
            nc.tensor.matmul(ps3[:, :], wc_s[:, :], h2T[:, w0:w0 + wd],
                             start=True, stop=True)
            ot = outp.tile([1, wd], F16, tag="ot")
            nc.scalar.activation(ot[:, :], ps3[:, :],
                                 mybir.ActivationFunctionType.Sigmoid,
                                 bias=bc_s[0:1, 0:1], scale=1.0)
            nc.sync.dma_start(out=p_out[0:1, w0:w0 + wd], in_=ot[:, :])

    nc.compile()
    return nc


def _build_nc(Bseg, seg_off, TOT, use_cc=True, ablate=(), single_packet=False):
    ab = set(ablate)
    nc = bacc.Bacc("TRN2", num_devices=C)
    TOT16 = TOT // 16
    TOTC = TOT // 128

    # ---- parameters ----
    p_idx = nc.declare_dram_parameter("idx_all", [128, TOT16], I16, isOutput=False)
    p_dc = nc.declare_dram_parameter("dc_all", [128, TOTC], F32, isOutput=False)
    p_iv = nc.declare_dram_parameter("iv_all", [128, TOTC], F32, isOutput=False)
    p_x = nc.declare_dram_parameter("x16", [NPC, 64], F16, isOutput=False)
    p_t5 = nc.declare_dram_parameter("timeT5", [5, NPC], F16, isOutput=False)
    p_uidx = nc.declare_dram_parameter("uidx", [128, NPC // 16], I16, isOutput=False)
    p_lidx = nc.declare_dram_parameter("lidx", [128, NPC // 16], I16, isOutput=False)
    p_ut = nc.declare_dram_parameter("utab", [10000, 128], F16, isOutput=False)
    p_lt = nc.declare_dram_parameter("ltab", [1000, 128], F16, isOutput=False)
    p_wt5 = nc.declare_dram_parameter("wt5", [5, 16], F16, isOutput=False)
    p_w = {}
    for nm in ("w1l", "w1r", "w2l", "w2r"):
        p_w[nm] = nc.declare_dram_parameter(nm, [128, 128], F16, isOutput=False)
    p_b1 = nc.declare_dram_parameter("b1", [128, 1], F32, isOutput=False)
    p_b2 = nc.declare_dram_parameter("b2", [128, 1], F32, isOutput=False)
    p_wc = nc.declare_dram_parameter("wc", [128, 1], F16, isOutput=False)
    p_bc = nc.declare_dram_parameter("bc", [1, 1], F32, isOutput=False)
    p_iota = nc.declare_dram_parameter("iota", [128, 128], F16, isOutput=False)
    p_out = nc.declare_dram_parameter("out", [1, NPC], F16, isOutput=True)

    cc_in = [nc.dram_tensor("cc_in0", [NPC, 128], F16),
             nc.dram_tensor("cc_in1", [NPC, 128], F16)]
    cc_out = [nc.dram_tensor("cc_out0", [NPAD, 128], F16, addr_space="Shared"),
              nc.dram_tensor("cc_out1", [NPAD, 128], F16, addr_space="Shared")]

    rg = [list(range(C))]

    # stage gather call layout
    L_sp = np.zeros((NSTAGES, NB), dtype=np.int64)
    O_sp = np.zeros((NSTAGES, NB), dtype=np.int64)
    for s in range(NSTAGES):
        for p in range(NB):
            L_sp[s, p] = Bseg[s * SG:(s + 1) * SG, p].sum()
            O_sp[s, p] = seg_off[s * SG, p]

    from contextlib import ExitStack

    with tile.TileContext(nc) as tc, ExitStack() as es:
        nc.gpsimd.load_library(mlp)
        _snap_cache = {}

        def snapv(v):
            if v not in _snap_cache:
                _snap_cache[v] = nc.gpsimd.snap(v)
            return _snap_cache[v]
        consts = es.enter_context(tc.tile_pool(name="consts", bufs=1))
        big = es.enter_context(tc.tile_pool(name="big", bufs=1))
        idxp = es.enter_context(tc.tile_pool(name="idxp", bufs=5))
        msgp = es.enter_context(tc.tile_pool(name="msgp", bufs=5))
        ohp = es.enter_context(tc.tile_pool(name="ohp", bufs=6))
        t5p = es.enter_context(tc.tile_pool(name="t5p", bufs=2))
        outp = es.enter_context(tc.tile_pool(name="outp", bufs=3))
        aggps = es.enter_context(tc.tile_pool(name="aggps", bufs=4, space="PSUM"))
        ps2p = es.enter_context(tc.tile_pool(name="ps2p", bufs=2, space="PSUM"))
        trp = es.enter_context(tc.tile_pool(name="trp", bufs=2, space="PSUM"))

        # ---- constants to SBUF ----
        iota_s = consts.tile_from(p_iota[:, :])
        wts = {nm: consts.tile_from(p_w[nm][:, :], name=nm) for nm in p_w}
        wt5_s = consts.tile_from(p_wt5[:, :])
        b1_s = consts.tile_from(p_b1[:, :])
        b2_s = consts.tile_from(p_b2[:, :])
        wc_s = consts.tile_from(p_wc[:, :])
        bc_s = consts.tile_from(p_bc[:, :])
        uidx_s = consts.tile_from(p_uidx[:, :])
        lidx_s = consts.tile_from(p_lidx[:, :])
        dc_s = consts.tile_from(p_dc[:, :])
        iv_s = consts.tile_from(p_iv[:, :])
        ident = consts.tile([128, 128], F16)
        make_identity(nc, ident[:, :])

        hT_cur = big.tile([128, NPC], F16, tag="hT0", name="hT0")
        aggT = big.tile([128, NPC], F16, tag="aggT")

        # ---- phase 0: build h0 (node-major) ----
        h_nm = big.tile([128, NG * 128], F16, tag="hnm")
        h3 = h_nm[:, :].rearrange("p (g d) -> p g d", d=128)
        # user emb gather straight into h_nm (table cols 64:96 hold the emb)
        nc.gpsimd.dma_gather(h3, p_ut[:, :], uidx_s[:, :], NPC, snapv(NPC), 128, single_packet=False)
        # loc emb gather to scratch (aggT slot reused), copy cols 96:112
        lg = big.tile([128, NG * 128], F16, tag="aggT")
        lg3 = lg[:, :].rearrange("p (g d) -> p g d", d=128)
        nc.gpsimd.dma_gather(lg3, p_lt[:, :], lidx_s[:, :], NPC, snapv(NPC), 128, single_packet=False)
        nc.vector.tensor_copy(h3[:, :, 96:112], lg3[:, :, 96:112])
        # x -> cols 0:64
        nc.sync.dma_start(
            out=h3[:, :, 0:64],
            in_=p_x[:, :].rearrange("(g p) d -> p g d", p=128),
        )
        # time mlp -> cols 112:128
        TCH, TNCH = _time_chunks()
        for t in range(TNCH):
            t5 = t5p.tile([5, TCH], F16, tag="t5")
            nc.sync.dma_start(out=t5[:, :], in_=p_t5[:, t * TCH:(t + 1) * TCH])
            for gi in range(TCH // 128):
                gg = t * (TCH // 128) + gi
                ps_t = trp.tile([128, 16], F32, tag="tr")
                nc.tensor.matmul(ps_t[:, :], t5[:, gi * 128:(gi + 1) * 128],
                                 wt5_s[:, :], start=True, stop=True)
                nc.vector.tensor_copy(h3[:, gg, 112:128], ps_t[:, :])
        # hT0 via PE transpose
        for gg in range(NG):
            ps_tr = trp.tile([128, 128], F16, tag="tr")
            nc.tensor.transpose(ps_tr[:, :], h3[:, gg, :], ident[:, :])
            nc.vector.tensor_copy(hT_cur[:, gg * 128:(gg + 1) * 128], ps_tr[:, :])
        # ship h0 to collective input
        nc.sync.dma_start(
            out=cc_in[0][:, :].rearrange("(g p) d -> p g d", p=128),
            in_=h3[:, :, :],
        )
        if use_cc:
            nc.gpsimd.collective_compute(
                "AllGather", mybir.AluOpType.bypass, replica_groups=rg,
                ins=[cc_in[0][:, :]], outs=[cc_out[0][:, :]],
            )
        else:
            nc.sync.dma_start(out=cc_out[0][0:NPC, :], in_=cc_in[0][:, :])

        # ---- conv layers ----
        for layer in range(2):
            wl = wts["w1l" if layer == 0 else "w2l"]
            wr = wts["w1r" if layer == 0 else "w2r"]
            bl = b1_s if layer == 0 else b2_s
            src_h = cc_out[layer]
            for s in range(NSTAGES):
                msgs = {}
                for p in range(NB):
                    L = int(L_sp[s, p])
                    if L == 0:
                        continue
                    O = int(O_sp[s, p])
                    it = idxp.tile([128, L // 16], I16, tag="idx")
                    if "idxdma" not in ab:
                        nc.sync.dma_start(out=it[:, :],
                                          in_=p_idx[:, O // 16:(O + L) // 16])
                    mt = msgp.tile([128, (L // 128) * 128], F16, tag="msg")
                    if "gather" not in ab:
                        nc.gpsimd.dma_gather(
                            mt[:, :].rearrange("p (b d) -> p b d", d=128),
                            src_h[p * W:(p + 1) * W, :],
                            it[:, :], L, snapv(L), 128,
                            single_packet=single_packet,
                        )
                    else:
                        nc.vector.tensor_copy(mt[:, 0:1], ident[:, 0:1])
                    msgs[p] = mt
                for gi in range(SG):
                    gg = s * SG + gi
                    nchunks = int(Bseg[gg, :].sum()) // 128
                    ps = aggps.tile([128, 128], F32, tag="agg")
                    k = 0
                    for p in range(NB):
                        nb_gp = int(Bseg[gg, p]) // 128
                        if nb_gp == 0:
                            continue
                        mt = msgs[p]
                        mcol0 = (int(seg_off[gg, p]) - int(O_sp[s, p])) // 128
                        ccol0 = int(seg_off[gg, p]) // 128
                        for cch in range(nb_gp):
                            oh = ohp.tile([128, 128], F16, tag="oh")
                            col = ccol0 + cch
                            if "onehot" not in ab:
                                nc.vector.tensor_scalar(
                                    oh[:, :], iota_s[:, :],
                                    dc_s[:, col:col + 1], iv_s[:, col:col + 1],
                                    op0=mybir.AluOpType.is_equal,
                                    op1=mybir.AluOpType.mult,
                                )
                            else:
                                nc.vector.tensor_copy(oh[:, 0:1], ident[:, 0:1])
                            mc = mcol0 + cch
                            if "mm" not in ab:
                                nc.tensor.matmul(
                                    ps[:, :], mt[:, mc * 128:(mc + 1) * 128],
                                    oh[:, :], start=(k == 0), stop=(k == nchunks - 1),
                                )
                            elif k == 0:
                                nc.tensor.matmul(
                                    ps[:, :], ident[:, :], ident[:, :],
                                    start=True, stop=True,
                                )
                            k += 1
                    nc.vector.tensor_copy(aggT[:, gg * 128:(gg + 1) * 128], ps[:, :])
            # dense: hT_next = relu(Wl^T aggT + Wr^T hT + b)
            hT_in = hT_cur
            hT_out = big.tile([128, NPC], F16,
                              tag="hT1" if layer == 0 else "hT0",
                              name=f"hTo{layer}")
            for (w0, wd) in _blocks():
                ps2 = ps2p.tile([128, wd], F32, tag="ps2", name="ps2")
                nc.tensor.matmul(ps2[:, :], wl[:, :], aggT[:, w0:w0 + wd],
                                 start=True, stop=False)
                nc.tensor.matmul(ps2[:, :], wr[:, :], hT_in[:, w0:w0 + wd],
                                 start=False, stop=True)
                nc.scalar.activation(hT_out[:, w0:w0 + wd], ps2[:, :],
                                     mybir.ActivationFunctionType.Relu,
                                     bias=bl[:, :], scale=1.0)
            if layer == 0:
                h_nm2 = big.tile([128, NG * 128], F16, tag="hnm")
                h23 = h_nm2[:, :].rearrange("p (g d) -> p g d", d=128)
                for gg in range(NG):
                    ps_tr = trp.tile([128, 128], F16, tag="tr")
                    nc.tensor.transpose(ps_tr[:, :],
                                        hT_out[:, gg * 128:(gg + 1) * 128],
                                        ident[:, :])
                    nc.vector.tensor_copy(h23[:, gg, :], ps_tr[:, :])
                nc.sync.dma_start(
                    out=cc_in[1][:, :].rearrange("(g p) d -> p g d", p=128),
                    in_=h23[:, :, :],
                )
                if use_cc:
                    nc.gpsimd.collective_compute(
                        "AllGather", mybir.AluOpType.bypass, replica_groups=rg,
                        ins=[cc_in[1][:, :]], outs=[cc_out[1][:, :]],
                    )
                else:
                    nc.sync.dma_start(out=cc_out[1][0:NPC, :], in_=cc_in[1][:, :])
            hT_cur = hT_out

        # ---- classifier ----
        h2T = hT_cur
        for (w0, wd) in _blocks():
            ps3 = ps2p.tile([1, wd], F32, tag="ps2", name="ps3")
            nc.tensor.matmul(ps3[:, :], wc_s[:, :], h2T[:, w0:w0 + wd],
                             start=True, stop=True)
            ot = outp.tile([1, wd], F16, tag="ot")
            nc.scalar.activation(ot[:, :], ps3[:, :],
                                 mybir.ActivationFunctionType.Sigmoid,
                                 bias=bc_s[0:1, 0:1], scale=1.0)
            nc.sync.dma_start(out=p_out[0:1, w0:w0 + wd], in_=ot[:, :])

    nc.compile()
    return nc


_CACHE = {}


def _pjrt_timed_runner(nc, n_cores):
    """Build a jitted SPMD executor for `nc` (same lowering path as
    bass_utils.run_bass_kernel_spmd -> bass2jax.run_bass_via_pjrt under axon),
    but with the jit object cached so repeat calls skip trace/compile.

    Returns run(in_maps, timed_iters) -> (results, exec_ns):
      results: list (per core) of {out_name: np.ndarray}
      exec_ns: min wall-ns of a warmed steady-state SPMD dispatch+execute
               (block_until_ready on device outputs; excludes H2D of inputs
               and NEFF compile).
    """
    import time as _time

    import jax
    from jax.experimental.shard_map import shard_map
    from jax.sharding import Mesh, NamedSharding, PartitionSpec

    from concourse import bass2jax
    from concourse.bass2jax import _bass_exec_p, partition_id_tensor

    bass2jax.install_neuronx_cc_hook()

    partition_name = nc.partition_id_tensor.name if nc.partition_id_tensor else None

    in_names = []
    out_names = []
    out_avals = []
    zero_outs = []
    for alloc in nc.m.functions[0].allocations:
        if not isinstance(alloc, mybir.MemoryLocationSet):
            continue
        name = alloc.memorylocations[0].name
        if alloc.kind == "ExternalInput":
            if name != partition_name:
                in_names.append(name)
        elif alloc.kind == "ExternalOutput":
            shape = tuple(alloc.tensor_shape)
            dtype = mybir.dt.np(alloc.dtype)
            out_avals.append(jax.core.ShapedArray(shape, dtype))
            zero_outs.append(np.zeros((n_cores * shape[0],) + shape[1:], dtype))
            out_names.append(name)
    n_params = len(in_names)
    n_outs = len(out_names)
    all_in_names = list(in_names) + list(out_names)
    if partition_name is not None:
        all_in_names.append(partition_name)
    donate = tuple(range(n_params, n_params + n_outs))

    def _body(*args):
        operands = list(args)
        if partition_name is not None:
            operands.append(partition_id_tensor())
        outs = _bass_exec_p.bind(
            *operands,
            out_avals=tuple(out_avals),
            in_names=tuple(all_in_names),
            out_names=tuple(out_names),
            lowering_input_output_aliases=(),
            sim_require_finite=True,
            sim_require_nnan=True,
            nc=nc,
        )
        return tuple(outs)

    devices = jax.devices()[:n_cores]
    mesh = Mesh(np.asarray(devices), ("core",))
    shard = NamedSharding(mesh, PartitionSpec("core"))
    in_specs = (PartitionSpec("core"),) * (n_params + n_outs)
    out_specs = (PartitionSpec("core"),) * n_outs
    sharded = jax.jit(
        shard_map(_body, mesh=mesh, in_specs=in_specs, out_specs=out_specs,
                  check_rep=False),
        donate_argnums=donate,
        keep_unused=True,
    )

    def run(in_maps, timed_iters=6):
        if nc.dbg_addr is not None:
            in_maps = [
                {**m, nc.dbg_addr.name: np.zeros((1, 2), np.uint32)}
                for m in in_maps
            ]
        concat_in = [
            np.concatenate([np.asarray(in_maps[c][name]) for c in range(n_cores)],
                           axis=0)
            for name in in_names
        ]
        in_dev = [jax.device_put(a, shard) for a in concat_in]
        jax.block_until_ready(in_dev)

        # warmup: triggers trace + NEFF compile + load; result reused as output
        zeros_dev = [jax.device_put(z, shard) for z in zero_outs]
        jax.block_until_ready(zeros_dev)
        out_arrs = sharded(*in_dev, *zeros_dev)
        jax.block_until_ready(out_arrs)

        # Timing: executions dispatched async pipeline through the axon
        # tunnel and serialize on-device, so the marginal cost of one more
        # execution is the true per-execution device time. Measure wall for
        # K1 and K2 back-to-back dispatch batches; slope = HW exec time
        # (tunnel round-trip latency cancels).
        def _batch_wall(k):
            zs = [[jax.device_put(z, shard) for z in zero_outs]
                  for _ in range(k)]
            for zl in zs:
                jax.block_until_ready(zl)
            t0 = _time.perf_counter()
            outs = [sharded(*in_dev, *zl) for zl in zs]
            jax.block_until_ready(outs)
            t1 = _time.perf_counter()
            return t1 - t0, outs

        K1, K2 = 4, 44
        exec_ns = None
        for _ in range(timed_iters):
            w1, _o1 = _batch_wall(K1)
            w2, _o2 = _batch_wall(K2)
            ns = int((w2 - w1) / (K2 - K1) * 1e9)
            exec_ns = ns if exec_ns is None else min(exec_ns, ns)

        results = []
        host = [np.asarray(a) for a in out_arrs]
        for c in range(n_cores):
            results.append(
                {name: host[i].reshape((n_cores,) + tuple(out_avals[i].shape))[c]
                 for i, name in enumerate(out_names)}
            )
        return results, exec_ns

    return run


def kernel(**inputs):
    x = np.asarray(inputs["x"], dtype=np.float32)
    edge_index = np.asarray(inputs["edge_index"])
    user_ids = np.asarray(inputs["user_ids"], dtype=np.int64)
    locations = np.asarray(inputs["locations"], dtype=np.int64)
    tf = np.asarray(inputs["time_features"], dtype=np.float32)

    B2, Braw, seg_off, L_all, TOT, core_arrays = _build_structure2(edge_index)

    key = ("nc2", TOT, tuple(B2.flatten().tolist()))
    if key not in _CACHE:
        _CACHE.clear()
        import os
        nc = _build_nc2(B2, Braw, seg_off, L_all, TOT,
                        use_cc=os.environ.get('NO_CC', '0') != '1')
        _CACHE[key] = (nc, _pjrt_timed_runner(nc, C))
    nc, runner = _CACHE[key]

    # shared (replicated) arrays
    ut = np.zeros((10000, 128), dtype=np.float16)
    ut[:, 64:96] = np.asarray(inputs["user_emb_table"], dtype=np.float32)
    lt = np.zeros((1000, 128), dtype=np.float16)
    lt[:, 96:112] = np.asarray(inputs["loc_emb_table"], dtype=np.float32)
    wt5 = np.concatenate(
        [np.asarray(inputs["W_time"], dtype=np.float32),
         np.asarray(inputs["b_time"], dtype=np.float32)[None, :]], axis=0
    ).astype(np.float16)
    iota = np.tile(np.arange(512, dtype=np.float16)[None, :], (128, 1))
    shared = {
        "utab": ut, "ltab": lt, "wt5": wt5, "iota": iota,
        "w1l": np.asarray(inputs["W1_l"], dtype=np.float16),
        "w1r": np.asarray(inputs["W1_r"], dtype=np.float16),
        "w2l": np.asarray(inputs["W2_l"], dtype=np.float16),
        "w2r": np.asarray(inputs["W2_r"], dtype=np.float16),
        "b1": np.asarray(inputs["b1"], dtype=np.float32).reshape(128, 1),
        "b2": np.asarray(inputs["b2"], dtype=np.float32).reshape(128, 1),
        "wc": np.asarray(inputs["Wc"], dtype=np.float16).reshape(128, 1),
        "bc": np.asarray(inputs["bc"], dtype=np.float32).reshape(1, 1),
    }

    in_maps = []
    for c in range(C):
        idx_arr, dc_arr, iv_arr = core_arrays[c]
        x16 = np.zeros((NPC, 64), dtype=np.float16)
        x16[:NV] = x[c * NV:(c + 1) * NV]
        t5 = np.ones((5, NPC), dtype=np.float16)
        t5[:4, :NV] = tf[c * NV:(c + 1) * NV].T
        t5[:4, NV:] = 0.0
        uid = np.zeros(NPC, dtype=np.int16)
        uid[:NV] = user_ids[c * NV:(c + 1) * NV]
        lid = np.zeros(NPC, dtype=np.int16)
        lid[:NV] = locations[c * NV:(c + 1) * NV]
        uidx = np.tile(uid.reshape(NPC // 16, 16).T, (8, 1))
        lidx = np.tile(lid.reshape(NPC // 16, 16).T, (8, 1))
        m = {
            "idx_all": idx_arr, "dc_all": dc_arr, "iv_all": iv_arr,
            "x16": x16, "timeT5": t5,
            "uidx": np.ascontiguousarray(uidx),
            "lidx": np.ascontiguousarray(lidx),
        }
        m.update(shared)
        in_maps.append(m)

    try:
        results, exec_ns = runner(in_maps)
        print(f"HW exec time: {exec_ns} ns")
    except Exception:
        import time as _time
        _t0 = _time.perf_counter()
        res = run_bass_kernel_spmd(nc, in_maps, list(range(C)))
        _t1 = _time.perf_counter()
        results = res.results
        print(f"HW exec time: {int((_t1 - _t0) * 1e9)} ns (wall of spmd call, upper bound)")
    out = np.zeros((N, 1), dtype=np.float32)
    for c in range(C):
        o = np.asarray(results[c]["out"], dtype=np.float32).reshape(NPC)
        out[c * NV:(c + 1) * NV, 0] = o[:NV]
    return out

